# revision 30
# baseline (speedup 1.0000x reference)
"""Deformable conv (DCNv2) + BN + ReLU Trainium2 Bass kernel.

Sharding: 8 cores = (2 batches) x (4 H-strips of 32 rows). Per core:
  1. 3x3 offset/mask conv via PE matmuls (bf16, pixel-major out, bias via
     ones-channel trick).
  2. Sample coords -> per-pixel patch index + 4 bilinear cell weights on DVE
     (f32, mod-based floor; zero-padded patch domain removes border masks).
  3. Chunked indirect DMA gathers of (c,cell)-packed 2x2x64ch bf16 patches
     (one 4-tile gather = 1 SWDGE instruction covering 36 taps/pixel).
  4. DVE applies cell weights (bf16 2x mode); PE identity-matmuls transpose
     + cell-accumulate to channel-major PSUM; main conv via 5 k-pair matmuls.
  5. BN partial sums via Act accum_out -> AllGather + local sum -> BN+ReLU.

Host side stages per-core inputs (bf16 slab, padded patch buffer, packed
weights, constants) and reassembles the output.
"""

import numpy as np
import concourse.bass as bass
import concourse.mybir as mybir
import concourse.tile as tile
from concourse.bass_utils import run_bass_kernel_spmd
from contextlib import ExitStack

F32 = mybir.dt.float32
BF16 = mybir.dt.bfloat16
I32 = mybir.dt.int32

B, C, O, H, W = 2, 64, 64, 128, 128
NCORES = 8
RPC = H // 4            # rows per core (4 strips per batch)
NPIX_TOT = B * H * W    # BN denominator
BN_EPS = 1e-5

K9 = 9
PADG = 4                # sampling-domain zero pad (offsets are within +-1.4)
GD = 2 * PADG + H + 2   # 138: padded canvas dim; patch grid is GD x GD
ELEM = 4 * C            # 2x2 cells x 64 ch per patch row
# tile-pair gather: one entry holds the same tap's patches for two adjacent
# rows; combo encodes the relative floor shift (dY in [-2,4], dX in [-3,3])
DYL, DXL = -2, -3
NDY, NDX = 7, 7
NCMB = NDY * NDX        # 49
NPROW = GD * GD * NCMB  # pair-table rows

CH = 4                  # phase-1/2 chunk size (tiles)


def _sap(ap, off_elems, dims):
    """AP with same tensor/partition dim, custom free dims."""
    return bass.AP(ap.tensor, ap.offset + off_elems, [ap.ap[0]] + dims)


def fix_multiwait(nc):
    """This env's walrus allows only ONE sem wait per instruction; split
    extras into single-wait drains on the same engine immediately before."""
    for f in nc.m.functions:
        for blk in f.blocks:
            i = 0
            while i < len(blk.instructions):
                ins = blk.instructions[i]
                si = ins.sync_info
                if si is not None and si.on_wait and len(si.on_wait) > 1:
                    waits = list(si.on_wait)
                    si.on_wait = [waits[-1]]
                    for j, w in enumerate(waits[:-1]):
                        d2 = mybir.InstDrain(
                            name=f"{ins.name}-wsplit{j}", ins=[], outs=[],
                            engine=ins.engine,
                        )
                        d2.sync_info = mybir.SyncInfo(on_wait=[w], on_update=[])
                        blk.instructions.insert(i, d2)
                        i += 1
                i += 1


def build_nc(T=RPC, collective=True, fixup=True, debug=False):
    nc = bass.Bass()

    xslab = nc.dram_tensor("xslab", [C + 1, T + 2, W + 2], BF16, kind="ExternalInput")
    pbp = nc.dram_tensor("pbp", [NPROW, 2 * ELEM], BF16, kind="ExternalInput")
    cy = nc.dram_tensor("cy", [128, T, K9], F32, kind="ExternalInput")
    cx = nc.dram_tensor("cx", [128, T, K9], F32, kind="ExternalInput")
    wcat = nc.dram_tensor("wcat", [C + 1, K9, 27], BF16, kind="ExternalInput")
    w2p = nc.dram_tensor("w2p", [128, 5, O], BF16, kind="ExternalInput")
    ident = nc.dram_tensor("ident", [128, 128], BF16, kind="ExternalInput")
    gb = nc.dram_tensor("gb", [O, 2], F32, kind="ExternalInput")
    outd = nc.dram_tensor("outn", [O, T, W], F32, kind="ExternalOutput")
    if debug:
        dbg_off = nc.dram_tensor("dbg_off", [128, T, 27], F32, kind="ExternalOutput")
        dbg_mk = nc.dram_tensor("dbg_mk", [128, T, K9], F32, kind="ExternalOutput")
        dbg_idx = nc.dram_tensor("dbg_idx", [128, T // 2, K9], I32, kind="ExternalOutput")
        dbg_w4 = nc.dram_tensor("dbg_w4", [128, T, K9, 4], F32, kind="ExternalOutput")
        dbg_opre = nc.dram_tensor("dbg_opre", [O, T, W], F32, kind="ExternalOutput")
        dbg_vch = nc.dram_tensor("dbg_vch", [128, 5, 128], F32, kind="ExternalOutput")
        dbg_g = nc.dram_tensor("dbg_g", [128, K9, C, 4], F32, kind="ExternalOutput")

    with tile.TileContext(nc) as tc:
        with ExitStack() as ctx:
            cpool = ctx.enter_context(tc.tile_pool(name="const", bufs=1))
            ppool = ctx.enter_context(tc.tile_pool(name="persist", bufs=1))
            wpool = ctx.enter_context(tc.tile_pool(name="wtmp", bufs=1))
            gpool = ctx.enter_context(tc.tile_pool(name="gath", bufs=3))
            prpool = ctx.enter_context(tc.tile_pool(name="prod", bufs=2))
            vpool = ctx.enter_context(tc.tile_pool(name="vch", bufs=2))
            sqpool = ctx.enter_context(tc.tile_pool(name="sqp", bufs=2))
            psA = ctx.enter_context(tc.tile_pool(name="psA", bufs=2, space="PSUM"))
            psB = ctx.enter_context(tc.tile_pool(name="psB", bufs=2, space="PSUM"))
            psC = ctx.enter_context(tc.tile_pool(name="psC", bufs=2, space="PSUM"))
            dpool = ctx.enter_context(tc.tile_pool(name="dram", bufs=1, space="DRAM"))

            TT = nc.vector.tensor_tensor
            TS = nc.vector.tensor_scalar
            STT = nc.vector.scalar_tensor_tensor
            AL = mybir.AluOpType
            AF = mybir.ActivationFunctionType

            # ---- prologue loads ----
            xs = cpool.tile([C + 1, T + 2, W + 2], BF16, tag="xs")
            nc.sync.dma_start(xs[:, 0:10, :], xslab[:, 0:10, :])
            nc.sync.dma_start(xs[:, 10:, :], xslab[:, 10:, :])
            wc = cpool.tile([C + 1, K9, 27], BF16, tag="wc")
            nc.sync.dma_start(wc[:], wcat[:])
            w2s = cpool.tile([128, 5, O], BF16, tag="w2s")
            nc.sync.dma_start(w2s[:], w2p[:])
            idt = cpool.tile([128, 128], BF16, tag="idt")
            nc.sync.dma_start(idt[:], ident[:])
            cys = cpool.tile([128, T, K9], F32, tag="cys")
            nc.sync.dma_start(cys[:], cy[:])
            cxs = cpool.tile([128, T, K9], F32, tag="cxs")
            nc.sync.dma_start(cxs[:], cx[:])
            gbs = cpool.tile([O, 2], F32, tag="gbs")
            nc.sync.dma_start(gbs[:], gb[:])
            epst = cpool.tile([128, 1], F32, tag="epst")
            nc.vector.memset(epst[:], BN_EPS)

            # ---- persistent tiles ----
            OFF = ppool.tile([128, T, 27], F32, tag="OFF")
            MK = ppool.tile([128, T, K9], F32, tag="MK")
            W4 = ppool.tile([128, T, K9, 4], BF16, tag="W4")
            IDXP = ppool.tile([128, T // 2, K9], I32, tag="IDXP")
            NG2 = T // 2
            ST1 = ppool.tile([O, NG2], F32, tag="ST1")
            ST2 = ppool.tile([O, NG2], F32, tag="ST2")
            OPRE = ppool.tile([O, T, W], BF16, tag="OPRE")
            ON = ppool.tile([O, T, W], F32, tag="ON")

            # ---- phase 1 (offset/mask conv) + phase 2 (coords), chunked;
            # small leading chunks let the first gathers start early ----
            CHUNKS = [2, 2] + [4] * ((T - 4) // 4)
            h0 = 0
            for CH in CHUNKS:
                pso = psA.tile([128, 4, 27], F32, tag="pso")
                for i in range(CH):
                    t = h0 + i
                    for k in range(K9):
                        ky, kx = k // 3, k % 3
                        lhsT = _sap(xs[:], (t + ky) * (W + 2) + kx, [[1, 128]])
                        rhs = _sap(wc[:], k * 27, [[1, 27]])
                        nc.tensor.matmul(pso[:, i, :], lhsT, rhs,
                                         start=(k == 0), stop=(k == K9 - 1))
                if debug:
                    nc.scalar.copy(OFF[:, h0:h0 + CH, :], pso[:, 0:CH, :])
                # sigmoid + phase 2 read the conv results straight from PSUM
                nc.scalar.activation(
                    _sap(MK[:], h0 * K9, [[K9, CH], [1, K9]]),
                    _sap(pso[:], 18, [[27, CH], [1, K9]]),
                    AF.Sigmoid)

                # phase 2 on [128, CH*K9] slices (f32, all on DVE)
                NF = CH * K9
                offy = _sap(pso[:], 0, [[27, CH], [2, K9]])
                offx = _sap(pso[:], 1, [[27, CH], [2, K9]])
                cysl = _sap(cys[:], h0 * K9, [[1, NF]])
                cxsl = _sap(cxs[:], h0 * K9, [[1, NF]])
                mks = _sap(MK[:], h0 * K9, [[1, NF]])

                def wt(tag, dt=F32):
                    tg = f"{tag}{CH}"
                    return wpool.tile([128, NF], dt, tag=tg, name=tg)

                py = wt("py"); TT(py[:], offy, cysl, AL.add)
                px = wt("px"); TT(px[:], offx, cxsl, AL.add)
                # floorfix y (DVE)
                riy = wt("riy", I32); nc.vector.tensor_copy(riy[:], py[:])
                rfy = wt("rfy"); nc.vector.tensor_copy(rfy[:], riy[:])
                gy = wt("gy"); TT(gy[:], rfy[:], py[:], AL.is_gt)
                Y0 = wt("Y0"); TT(Y0[:], rfy[:], gy[:], AL.subtract)
                WY = wt("WY"); TT(WY[:], py[:], Y0[:], AL.subtract)
                # floorfix x
                rix = wt("rix", I32); nc.vector.tensor_copy(rix[:], px[:])
                rfx = wt("rfx"); nc.vector.tensor_copy(rfx[:], rix[:])
                gx = wt("gx"); TT(gx[:], rfx[:], px[:], AL.is_gt)
                X0 = wt("X0"); TT(X0[:], rfx[:], gx[:], AL.subtract)
                WX = wt("WX"); TT(WX[:], px[:], X0[:], AL.subtract)
                # tile-pair composite index (even tile t, odd tile t+1):
                #   comp = (Y0e*GD + X0e)*NCMB + (Y0o_s - Y0e)*NDX + (X0o_s - X0e)
                # where the odd tiles' cys/cxs carry +(-DYL)/+(-DXL) shifts.
                NH2 = CH // 2
                NF2 = NH2 * K9
                evn = lambda tl: _sap(tl, 0, [[2 * K9, NH2], [1, K9]])
                odd = lambda tl: _sap(tl, K9, [[2 * K9, NH2], [1, K9]])
                idxe = wpool.tile([128, NF2], F32, tag="idxe", name="idxe")
                STT(idxe[:], evn(Y0[:]), float(GD), evn(X0[:]), AL.mult, AL.add)
                dY = wpool.tile([128, NF2], F32, tag="dY", name="dY")
                TT(dY[:], odd(Y0[:]), evn(Y0[:]), AL.subtract)
                dX = wpool.tile([128, NF2], F32, tag="dX", name="dX")
                TT(dX[:], odd(X0[:]), evn(X0[:]), AL.subtract)
                zc = wpool.tile([128, NF2], F32, tag="zc", name="zc")
                STT(zc[:], dY[:], float(NDX), dX[:], AL.mult, AL.add)
                comp = wpool.tile([128, NF2], F32, tag="comp", name="comp")
                STT(comp[:], idxe[:], float(NCMB), zc[:], AL.mult, AL.add)
                nc.vector.tensor_copy(
                    _sap(IDXP[:], (h0 // 2) * K9, [[1, NF2]]), comp[:])

                V1 = wt("V1"); TT(V1[:], WY[:], mks, AL.mult)
                V0 = wt("V0"); TT(V0[:], mks, V1[:], AL.subtract)
                # W4 cells, bf16 out; cell order (00,01,10,11):
                #   c01 = V0*WX, c11 = V1*WX, c00 = V0-c01, c10 = V1-c11
                w4c = lambda cell: _sap(W4[:], h0 * K9 * 4 + cell, [[4, NF]])
                TT(w4c(1), V0[:], WX[:], AL.mult)
                TT(w4c(3), V1[:], WX[:], AL.mult)
                TT(w4c(0), V0[:], w4c(1), AL.subtract)
                TT(w4c(2), V1[:], w4c(3), AL.subtract)
                h0 += CH

            # ---- main loop: tile-pair gathers (one SWDGE index per
            # partition; each entry = same tap's patches for rows t, t+1) ----
            for tp in range(T // 2):
                Gp = gpool.tile([128, K9, 2, C, 4], BF16, tag="gp")
                for k in range(K9):
                    nc.gpsimd.indirect_dma_start(
                        Gp[:, k, :, :, :].rearrange("p a b c -> p (a b c)"),
                        None, pbp[:],
                        bass.IndirectOffsetOnAxis(
                            ap=IDXP[:, tp, k:k + 1], axis=0),
                    )
                if True:
                    vch = vpool.tile([128, 2, 5, 128], BF16, tag="vch")
                    for i2 in range(2):
                        t = tp * 2 + i2
                        prod = prpool.tile([128, K9, C, 4], BF16, tag="prod")
                        for kk in range(5):
                            k0, nk = 2 * kk, (2 if kk < 4 else 1)
                            w4b = _sap(W4[:], (t * K9 + k0) * 4,
                                       [[4, nk], [0, C], [1, 4]])
                            gsl = _sap(Gp[:], (2 * k0 + i2) * C * 4,
                                       [[2 * C * 4, nk], [4, C], [1, 4]])
                            TT(prod[:, k0:k0 + nk, :, :], gsl, w4b, AL.mult)
                        psTa = psA.tile([128, 5, 128], F32, tag="psT")
                        for kk in range(4):
                            for cell in range(4):
                                lhsT = _sap(prod[:], 2 * kk * C * 4 + cell,
                                            [[C * 4, 2], [4, C]])
                                nc.tensor.matmul(psTa[:, kk, :], lhsT, idt[:],
                                                 start=(cell == 0), stop=(cell == 3))
                        for cell in range(4):
                            lhsT = _sap(prod[:], 8 * C * 4 + cell, [[4, C]])
                            nc.tensor.matmul(psTa[0:O, 4, :], lhsT, idt[:],
                                             start=(cell == 0), stop=(cell == 3))
                        nc.scalar.copy(vch[:, i2, :, :], psTa[:])
                        if debug and tp == 0 and i2 == 0:
                            gdbg = ppool.tile([128, K9, C, 4], F32, tag="gdbg")
                            nc.vector.tensor_copy(gdbg[:], gsl)
                            nc.sync.dma_start(dbg_g[:], gdbg[:])
                            vdbg = ppool.tile([128, 5, 128], F32, tag="vdbg")
                            nc.vector.tensor_copy(vdbg[:], psTa[:])
                            nc.sync.dma_start(dbg_vch[:], vdbg[:])
                    # main conv for this 2-tile pair
                    po = psB.tile([O, 2, 128], F32, tag="po")
                    for kk in range(4):
                        nc.tensor.matmul(po[:], w2s[:, kk, :], vch[:, :, kk, :],
                                         start=(kk == 0), stop=False)
                    nc.tensor.matmul(po[:], w2s[0:O, 4, :], vch[0:O, :, 4, :],
                                     start=False, stop=True)
                    t2 = tp * 2
                    nc.scalar.activation(OPRE[:, t2:t2 + 2, :], po[:], AF.Copy,
                                         accum_out=ST1[:, tp:tp + 1])
                    sqj = sqpool.tile([O, 2, 128], BF16, tag="sqj")
                    nc.scalar.activation(sqj[:], po[:], AF.Square,
                                         accum_out=ST2[:, tp:tp + 1])

            if debug:
                nc.sync.dma_start(dbg_off[:], OFF[:])
                nc.sync.dma_start(dbg_mk[:], MK[:])
                nc.sync.dma_start(dbg_idx[:], IDXP[:])
                w4f = ppool.tile([128, T, K9, 4], F32, tag="w4f")
                nc.vector.tensor_copy(w4f[:], W4[:])
                nc.sync.dma_start(dbg_w4[:], w4f[:])
                opref = ppool.tile([O, T, W], F32, tag="opref")
                nc.vector.tensor_copy(opref[:], OPRE[:])
                nc.sync.dma_start(dbg_opre[:], opref[:])

            # ---- BN tail ----
            s1 = ppool.tile([O, 2], F32, tag="s1")
            nc.vector.tensor_reduce(s1[:, 0:1], ST1[:], mybir.AxisListType.X, AL.add)
            nc.vector.tensor_reduce(s1[:, 1:2], ST2[:], mybir.AxisListType.X, AL.add)
            if collective:
                cin = dpool.tile([O, 2], F32, tag="cin")
                cout = dpool.tile([NCORES * O, 2], F32, tag="cout")
                nc.sync.dma_start(cin[:], s1[:])
                nc.gpsimd.collective_compute(
                    "AllGather", AL.bypass,
                    replica_groups=[list(range(NCORES))],
                    ins=[cin.opt()], outs=[cout.opt()],
                )
                sg8 = ppool.tile([O, 2, NCORES], F32, tag="sg8")
                csrc = bass.AP(cout[:].tensor, cout[:].offset,
                               [[2, O], [1, 2], [2 * O, NCORES]])
                nc.sync.dma_start(sg8[:], csrc)
                sg = ppool.tile([O, 2], F32, tag="sg")
                nc.vector.tensor_reduce(sg[:], sg8[:], mybir.AxisListType.X, AL.add)
                denom = float(NPIX_TOT)
            else:
                sg = s1
                denom = float(T * W)

            mv = ppool.tile([O, 2], F32, tag="mv")
            TS(mv[:], sg[:], 1.0 / denom, None, AL.mult)
            mean = mv[:, 0:1]
            m2 = ppool.tile([O, 1], F32, tag="m2")
            TT(m2[:], mean, mean, AL.mult)
            var = ppool.tile([O, 1], F32, tag="var")
            TT(var[:], mv[:, 1:2], m2[:], AL.subtract)
            stdt = ppool.tile([O, 1], F32, tag="stdt")
            nc.scalar.activation(stdt[:], var[:], AF.Sqrt, bias=epst[0:O, :])
            rstd = ppool.tile([O, 1], F32, tag="rstd")
            nc.vector.reciprocal(rstd[:], stdt[:])
            scl = ppool.tile([O, 1], F32, tag="scl")
            TT(scl[:], gbs[:, 0:1], rstd[:], AL.mult)
            msc = ppool.tile([O, 1], F32, tag="msc")
            TT(msc[:], mean, scl[:], AL.mult)
            sh = ppool.tile([O, 1], F32, tag="sh")
            TT(sh[:], gbs[:, 1:2], msc[:], AL.subtract)

            NQ = T // 4
            TSP = nc.vector.tensor_scalar
            for q in range(4):
                q0 = q * NQ
                if q < 3:
                    nc.scalar.activation(ON[:, q0:q0 + NQ, :], OPRE[:, q0:q0 + NQ, :],
                                         AF.Relu, bias=sh[:], scale=scl[:])
                else:
                    TSP(ON[:, q0:q0 + NQ, :], OPRE[:, q0:q0 + NQ, :],
                        scl[:], sh[:], AL.mult, AL.add)
                    TSP(ON[:, q0:q0 + NQ, :], ON[:, q0:q0 + NQ, :],
                        0.0, None, AL.max)
                nc.sync.dma_start(outd[:, q0:q0 + NQ, :], ON[:, q0:q0 + NQ, :])

    if fixup:
        fix_multiwait(nc)
    return nc


# ---------------- host-side preparation ----------------

def _host_prep(x, conv_w, off_w, off_b, mask_w, mask_b, gamma, beta, T=RPC):
    """Build the 8 per-core input maps."""
    import ml_dtypes
    BF = ml_dtypes.bfloat16
    x = np.asarray(x, np.float32)

    # padded canvas (zero outside borders) in channels-last
    xcl = np.transpose(x, (0, 2, 3, 1))  # [B, H, W, C]
    canvas = np.zeros((B, GD, GD, C), np.float32)
    canvas[:, PADG:PADG + H, PADG:PADG + W] = xcl

    # base patches: PB[b, y, x, :] = 2x2 patch with top-left (y, x),
    # packed (c, cell) with cell=(dy*2+dx).
    PB = np.zeros((B, GD, GD, ELEM), BF)
    S = GD - 1  # 137 valid start positions per dim
    cells = [canvas[:, dy:dy + S, dx:dx + S, :]
             for dy in range(2) for dx in range(2)]
    patch = np.stack(cells, axis=4)  # [B, 137, 137, C, 4]
    PB[:, :S, :S] = patch.reshape(B, S, S, ELEM).astype(BF)
    del patch, cells, canvas

    # tile-pair table: PBP[b, y, x, combo] = PB[b, y, x] ++ PB[b, y+dY, x+dX]
    # with combo = (dY - DYL)*NDX + (dX - DXL).
    PBP = np.zeros((B, GD, GD, NCMB, 2 * ELEM), BF)
    PBP[:, :, :, :, :ELEM] = PB[:, :, :, None, :]
    for dY in range(DYL, DYL + NDY):
        for dX in range(DXL, DXL + NDX):
            cmb = (dY - DYL) * NDX + (dX - DXL)
            ylo, yhi = max(0, -dY), min(GD, GD - dY)
            xlo, xhi = max(0, -dX), min(GD, GD - dX)
            PBP[:, ylo:yhi, xlo:xhi, cmb, ELEM:] = \
                PB[:, ylo + dY:yhi + dY, xlo + dX:xhi + dX, :]

    # wcat: [C+1, 9, 27]; ones row = biases at k=0
    wfull = np.concatenate([off_w, mask_w], axis=0)  # [27, C, 3, 3]
    wcat = np.zeros((C + 1, K9, 27), np.float32)
    wcat[:C] = np.transpose(wfull.reshape(27, C, K9), (1, 2, 0))
    bias = np.concatenate([off_b, mask_b]).astype(np.float32)
    wcat[C, 0, :] = bias
    wcat = wcat.astype(BF)

    # w2p [128, 5, O]: rows (kq*64+c) -> w2[c, 2kk+kq, o]; kk=4 rows 64+ = 0
    w2 = np.transpose(conv_w.reshape(O, C, K9), (1, 2, 0)).astype(np.float32)  # [C,9,O]
    w2p = np.zeros((128, 5, O), np.float32)
    for kk in range(4):
        for kq in range(2):
            w2p[kq * C:(kq + 1) * C, kk, :] = w2[:, 2 * kk + kq, :]
    w2p[0:C, 4, :] = w2[:, 8, :]
    w2p = w2p.astype(BF)

    ident = np.eye(128, dtype=np.float32).astype(BF)
    gb = np.stack([np.asarray(gamma, np.float32), np.asarray(beta, np.float32)], axis=1)

    ky = np.repeat(np.arange(3), 3).astype(np.float32)
    kx = np.tile(np.arange(3), 3).astype(np.float32)
    gx = np.arange(128, dtype=np.float32)

    in_maps = []
    for core in range(NCORES):
        b, strip = divmod(core, 4)
        r0 = strip * RPC
        xslab = np.zeros((C + 1, T + 2, W + 2), np.float32)
        lo, hi = r0 - 1, r0 + T + 1
        glo, ghi = max(lo, 0), min(hi, H)
        xslab[:C, (glo - lo):(ghi - lo), 1:W + 1] = x[b, :, glo:ghi, :]
        xslab[C] = 1.0
        # odd tiles carry the combo-shift so the composite index needs no
        # extra constant: Y0odd_shifted - Y0even = dY - DYL etc.
        oshift = (np.arange(T) % 2 == 1).astype(np.float32)
        cyv = (r0 + np.arange(T)[None, :, None] + (ky - 1.0 + PADG)[None, None, :]
               + (-DYL) * oshift[None, :, None] + np.zeros((128, 1, 1))).astype(np.float32)
        cxv = (gx[:, None, None] + (kx - 1.0 + PADG)[None, None, :]
               + (-DXL) * oshift[None, :, None] + np.zeros((1, 1, 1))).astype(np.float32)
        in_maps.append({
            "xslab": xslab.astype(BF), "pbp": PBP[b].reshape(NPROW, 2 * ELEM),
            "cy": cyv, "cx": cxv, "wcat": wcat, "w2p": w2p,
            "ident": ident, "gb": gb,
        })
    return in_maps


_NC_CACHE = {}


def kernel(x, conv_w, off_w, off_b, mask_w, mask_b, gamma, beta):
    if "nc" not in _NC_CACHE:
        _NC_CACHE["nc"] = build_nc()
    nc = _NC_CACHE["nc"]
    in_maps = _host_prep(x, conv_w, off_w, off_b, mask_w, mask_b, gamma, beta)
    res = run_bass_kernel_spmd(nc, in_maps, core_ids=list(range(NCORES)))
    out = np.zeros((B, O, H, W), np.float32)
    for core in range(NCORES):
        b, strip = divmod(core, 4)
        r0 = strip * RPC
        out[b, :, r0:r0 + RPC, :] = res.results[core]["outn"]
    return out
